# revision 31
# baseline (speedup 1.0000x reference)
"""Trainium2 Bass kernel for nn_Attention_33157147525297.

Graph-mixed multi-head attention, B=64, N=196 tokens, D=768, H=12 heads.
Data-parallel over batch: 8 batches per NeuronCore x 8 cores.

Math restructuring (host side):
  reference: attn = softmax(G @ (q k^T * scale)); out = attn @ v
  G mixes the query index only, so G @ (q k^T) == (G q) k^T.  Folding the
  1/8 scale into G gives q' = (scale*G) @ q = ((scale*G) @ x) @ Wq^T, so the
  whole graph-mix collapses into a pre-mix of x on the query path: xg = G_s @ x.

Device pipeline per core (all matmuls bf16, PSUM f32):
  A: [x^T | xg^T] = x[b]^T @ [I | G_s^T]      (TensorE transpose + graph mix)
  B: q'^T = Wq @ xg^T, k^T = Wk @ x^T (feature-major); v = x @ Wv^T (token-major)
  C: per (b,h): S^T = k q'^T (both j-tiles in one psum bank); P^T = exp(S^T)
     in one ACT op; O^T = v^T P^T; softmax sums via ones-matmul into the same
     bank (sequential accumulation groups — interleaving two open groups in
     one psum bank corrupts the first); 1/sums broadcast via K=1 matmul.
  D: y = O_full @ Wp^T + b  (token-major, direct DMA out)

Infra notes: this container's walrus accepts only ONE attached semaphore
wait per instruction — _install_wait_split() hoists extra waits onto
standalone EventSemaphore instructions.  Timing feedback came from the
concourse cost-model TimelineSim (NTFF profiling hooks are unavailable
under this axon client); predicted single-core exec ~206 us.
"""
import os
import sys
import numpy as np
import ml_dtypes

sys.path.insert(0, "/opt/trn_rl_repo")

SIZE, N_TOK, DIM, HEADS, HEAD_DIM, BATCH = 14, 196, 768, 12, 64, 64
N_CORES = 8
B_PER_CORE = BATCH // N_CORES  # 8
NT2 = 2 * N_TOK  # 392
BF16 = ml_dtypes.bfloat16

# token-dim partition tiles (196 = 128 + 68)
TOK_TILES = [(0, 128), (128, 68)]

LAST_EXEC_NS = None
LAST_TRACE = None


def _grid_g(factors):
    idx = np.arange(SIZE * SIZE).reshape(SIZE, SIZE)
    A = np.zeros((N_TOK, N_TOK), dtype=np.float32)
    for di, dj in [(-1, 0), (1, 0), (0, -1), (0, 1)]:
        for i in range(SIZE):
            for j in range(SIZE):
                ii, jj = i + di, j + dj
                if 0 <= ii < SIZE and 0 <= jj < SIZE:
                    A[idx[i, j], idx[ii, jj]] = 1.0
    NN = A / (A.sum(axis=1, keepdims=True) + 1.0)
    C = np.eye(N_TOK, dtype=np.float32) / 2.0
    return factors[0] * C + factors[1] * NN


def _install_wait_split():
    """This container's walrus rejects >1 attached semaphore wait per
    instruction ("Too many sync wait commands").  Hoist excess waits onto
    standalone InstEventSemaphore instructions just before, on the same
    engine — engine queues are in-order, so semantics are identical."""
    import concourse.mybir as mybir
    import concourse.tile as tile
    from concourse.vector_clock import ScopedClock

    TC = tile.TileContext
    if getattr(TC, "_wait_split_patched", False):
        return
    LIMIT = 1

    def _split(tc, inst):
        si = inst.sync_info
        if (si is None or not si.on_wait or len(si.on_wait) <= LIMIT
                or inst.engine == mybir.EngineType.Unassigned):
            return
        waits = list(si.on_wait)
        extra, keep = waits[:-LIMIT], waits[-LIMIT:]
        for i, w in enumerate(extra):
            ev = mybir.InstEventSemaphore(
                name=f"{inst.name}-ws{i}", engine=inst.engine,
                sync_info=mybir.SyncInfo(on_wait=[w], on_update=[]),
            )
            tc._add_instruction(ev)
        inst.sync_info = mybir.SyncInfo(on_wait=keep,
                                        on_update=list(si.on_update))

    orig_commit = TC._commit_instruction

    def patched_commit(self, inst, lazy_reg_writes=True):
        _split(self, inst)
        return orig_commit(self, inst, lazy_reg_writes=lazy_reg_writes)

    TC._commit_instruction = patched_commit

    def patched_drain_and_barrier(self, tick_clock, wait_clock):
        nc = self.nc
        probe = mybir.InstNoOp(
            name=f"drain-probe-{nc.next_id()}", engine=mybir.EngineType.SP)
        wait_clock.add_sem_waits(
            probe, ScopedClock({None: tick_clock.global_clock}))
        pw = probe.sync_info.on_wait if probe.sync_info else []
        for i, w in enumerate(pw):
            ev = mybir.InstEventSemaphore(
                name=f"drainw-{nc.next_id()}-{i}", engine=mybir.EngineType.SP,
                sync_info=mybir.SyncInfo(on_wait=[w], on_update=[]),
            )
            self._add_instruction(ev)
        nc.sync.drain()
        nc.all_engine_barrier()
        assert self.sems is not None
        popped = nc._tile_sem_poison_stack.pop()
        assert popped is self._sem_poison
        nc.clear_and_free_semaphores(list(self.sems.allocated().values()))
        nc.all_engine_barrier()

    TC._drain_and_barrier = patched_drain_and_barrier
    TC._wait_split_patched = True


def _build_bass():
    import concourse.bass as bass
    import concourse.mybir as mybir
    import concourse.tile as tile

    _install_wait_split()

    f32 = mybir.dt.float32
    bf16 = mybir.dt.bfloat16
    AF = mybir.ActivationFunctionType

    nc = bass.Bass()

    x_d = nc.declare_dram_parameter("x", [B_PER_CORE, N_TOK, DIM], bf16, isOutput=False)
    ig_d = nc.declare_dram_parameter("ig", [N_TOK, NT2], bf16, isOutput=False)
    wq_d = nc.declare_dram_parameter("wqT", [DIM, DIM], bf16, isOutput=False)
    wk_d = nc.declare_dram_parameter("wkT", [DIM, DIM], bf16, isOutput=False)
    wv_d = nc.declare_dram_parameter("wvT", [DIM, DIM], bf16, isOutput=False)
    wp_d = nc.declare_dram_parameter("wpT", [DIM, DIM], bf16, isOutput=False)
    bias_d = nc.declare_dram_parameter("bias", [DIM], f32, isOutput=False)
    out_d = nc.declare_dram_parameter(
        "out", [B_PER_CORE, N_TOK, DIM], f32, isOutput=True
    )

    with tile.TileContext(nc) as tc:
        with (
            tc.tile_pool(name="const", bufs=1) as const_p,
            tc.tile_pool(name="big", bufs=1) as big_p,
            tc.tile_pool(name="cp", bufs=7) as cp_p,
            tc.tile_pool(name="ps_big", bufs=2, space="PSUM") as ps_big,
            tc.tile_pool(name="ps_s", bufs=2, space="PSUM") as ps_s,
            tc.tile_pool(name="ps_pv", bufs=3, space="PSUM") as ps_pv,
            tc.tile_pool(name="ps_sc", bufs=1, space="PSUM") as ps_sc,
        ):
            # ---- constants (x + ig first so stage A starts ASAP;
            #      weights are only needed from stage B on) ----
            ig_sb = []
            for ti, (t0, tsz) in enumerate(TOK_TILES):
                t = const_p.tile([128, NT2], bf16, name=f"ig{ti}")
                nc.sync.dma_start(out=t[:tsz], in_=ig_d[t0:t0 + tsz, :])
                ig_sb.append(t)

            ones_col = const_p.tile([128, 1], bf16, name="ones_col")
            nc.vector.memset(ones_col, 1.0)
            ones_row = const_p.tile([128, 64], bf16, name="ones_row")
            nc.vector.memset(ones_row, 1.0)

            # ---- persistent activations ----
            NTB = N_TOK * B_PER_CORE  # 1568
            x_sb = [
                [big_p.tile([128, DIM], bf16, name=f"x{b}_{ti}") for ti in range(2)]
                for b in range(B_PER_CORE)
            ]
            # xxg[kt]: cols 0:1568 = x^T, cols 1568:3136 = xg^T
            XGOFF = NTB
            xxg_sb = [big_p.tile([128, 2 * NTB], bf16, name=f"xxg{k}")
                      for k in range(6)]
            qT_sb = [big_p.tile([128, NTB], bf16, name=f"qT{k}")
                     for k in range(6)]
            kT_sb = [big_p.tile([128, NTB], bf16, name=f"kT{k}")
                     for k in range(6)]
            v_sb = [
                [big_p.tile([128, DIM], bf16, name=f"v{b}_{ti}") for ti in range(2)]
                for b in range(B_PER_CORE)
            ]
            o_sb = [big_p.tile([128, NTB], bf16, name=f"o{k}")
                    for k in range(6)]

            # ---- stage A: load x; [x^T | xg^T] per batch ----
            for b in range(B_PER_CORE):
                for ti, (t0, tsz) in enumerate(TOK_TILES):
                    nc.sync.dma_start(out=x_sb[b][ti][:tsz],
                                      in_=x_d[b, t0:t0 + tsz, :])

            # weights after x so stage A's inputs land first
            def load_w(d, nm):
                ts = []
                for kt in range(6):
                    t = const_p.tile([128, DIM], bf16, name=f"{nm}{kt}")
                    nc.sync.dma_start(out=t, in_=d[kt * 128:(kt + 1) * 128, :])
                    ts.append(t)
                return ts

            wq_sb = load_w(wq_d, "wq")
            wk_sb = load_w(wk_d, "wk")
            wv_sb = load_w(wv_d, "wv")
            wp_sb = load_w(wp_d, "wp")
            bias_sb = const_p.tile([128, DIM], f32, name="bias")
            nc.sync.dma_start(out=bias_sb,
                              in_=bias_d[None, :].broadcast_to([128, DIM]))

            for b in range(B_PER_CORE):
                for mt in range(6):
                    ps = ps_big.tile([128, NT2], f32, tag="psA")
                    for ti, (t0, tsz) in enumerate(TOK_TILES):
                        nc.tensor.matmul(
                            ps, x_sb[b][ti][:tsz, mt * 128:(mt + 1) * 128],
                            ig_sb[ti][:tsz], start=(ti == 0), stop=(ti == 1),
                        )
                    c0 = b * N_TOK
                    # one copy for both halves: dst [128,2,196] strided by NTB
                    dst = xxg_sb[mt].rearrange("p (g c) -> p g c", g=2)[
                        :, :, c0:c0 + N_TOK]
                    srcv = ps.rearrange("p (g c) -> p g c", g=2)
                    nc.vector.tensor_copy(dst, srcv)

            # ---- stage B: q'^T, k^T (feature-major); v (token-major) ----
            for dst, w, goff in ((qT_sb, wq_sb, NTB), (kT_sb, wk_sb, 0)):
                for mt in range(6):
                    for nt in range(4):
                        ps = ps_big.tile([128, NT2], f32, tag="psA")
                        for kt in range(6):
                            nc.tensor.matmul(
                                ps, w[kt][:, mt * 128:(mt + 1) * 128],
                                xxg_sb[kt][:, goff + nt * NT2:goff + (nt + 1) * NT2],
                                start=(kt == 0), stop=(kt == 5),
                            )
                        nc.vector.tensor_copy(
                            dst[mt][:, nt * NT2:(nt + 1) * NT2], ps)
            for b in range(B_PER_CORE):
                for ti, (t0, tsz) in enumerate(TOK_TILES):
                    for nt in range(2):
                        ps = ps_big.tile([128, NT2], f32, tag="psA")
                        for kt in range(6):
                            nc.tensor.matmul(
                                ps[:tsz, :384],
                                xxg_sb[kt][:, b * N_TOK + t0:b * N_TOK + t0 + tsz],
                                wv_sb[kt][:, nt * 384:(nt + 1) * 384],
                                start=(kt == 0), stop=(kt == 5),
                            )
                        nc.scalar.activation(
                            v_sb[b][ti][:tsz, nt * 384:(nt + 1) * 384],
                            ps[:tsz, :384], AF.Copy)

            # ---- stage C: attention per (batch, head-pair) ----
            # Per head: S^T for both j-tiles lands in ONE psum bank
            # ([:,0:196]=j-tile0, [:68,196:392]=j-tile1), one exp covers both.
            # Per pair: PV output (rows hb:hb+64 per head, cols 0:196) and the
            # ones-matmul softmax sums (row hb, cols 196:392) share one bank —
            # the two accumulation groups must stay SEQUENTIAL (interleaving
            # two open groups in one psum bank corrupts the first).
            for b in range(B_PER_CORE):
                c0 = b * N_TOK
                for p in range(6):  # head pair: heads 2p (rows 0:64), 2p+1 (64:128)
                    pv_ps = ps_pv.tile([128, NT2], f32, tag="pv")
                    sums_ps = ps_sc.tile([128, N_TOK], f32, tag="sc")
                    ssb = cp_p.tile([128, N_TOK], bf16, tag="ssb")
                    for hi in range(2):
                        hb = hi * 64
                        s_ps = ps_s.tile([128, NT2], f32, tag="s")
                        for ti, (t0, tsz) in enumerate(TOK_TILES):
                            nc.tensor.matmul(
                                s_ps[:tsz, ti * N_TOK:(ti + 1) * N_TOK],
                                kT_sb[p][hb:hb + 64, c0 + t0:c0 + t0 + tsz],
                                qT_sb[p][hb:hb + 64, c0:c0 + N_TOK],
                                start=True, stop=True,
                            )
                        pT = cp_p.tile([128, NT2], bf16, tag="pT")
                        nc.scalar.activation(pT, s_ps, AF.Exp)
                        for ti, (t0, tsz) in enumerate(TOK_TILES):
                            nc.tensor.matmul(
                                pv_ps[hb:hb + 64, 0:N_TOK],
                                v_sb[b][ti][:tsz, 2 * p * 64 + hb:2 * p * 64 + hb + 64],
                                pT[:tsz, ti * N_TOK:(ti + 1) * N_TOK],
                                start=(ti == 0), stop=(ti == 1),
                                tile_position=(0, hb),
                            )
                        for ti, (t0, tsz) in enumerate(TOK_TILES):
                            nc.tensor.matmul(
                                pv_ps[hb:hb + 1, N_TOK:NT2],
                                ones_col[:tsz],
                                pT[:tsz, ti * N_TOK:(ti + 1) * N_TOK],
                                start=(ti == 0), stop=(ti == 1),
                                tile_position=(0, hb),
                            )
                        # raw sums -> bf16 sbuf row, broadcast to 64 rows
                        nc.scalar.activation(ssb[hb:hb + 1, 0:N_TOK],
                                             pv_ps[hb:hb + 1, N_TOK:NT2],
                                             AF.Copy)
                        nc.tensor.matmul(
                            sums_ps[hb:hb + 64, :],
                            ones_row[hb:hb + 1, :],
                            ssb[hb:hb + 1, 0:N_TOK],
                            start=True, stop=True,
                            tile_position=(hb, hb),
                        )
                    scale_sb = cp_p.tile([128, N_TOK], bf16, tag="scale")
                    with nc.allow_low_precision(reason="softmax recip bf16"):
                        nc.vector.reciprocal(scale_sb, sums_ps)
                    nc.vector.tensor_mul(o_sb[p][:, c0:c0 + N_TOK],
                                         pv_ps[:, 0:N_TOK], scale_sb)

            # ---- stage D: y = O @ Wp^T + bias; DMA out ----
            for b in range(B_PER_CORE):
                c0 = b * N_TOK
                for ti, (t0, tsz) in enumerate(TOK_TILES):
                    for nt in range(2):
                        ps = ps_big.tile([128, NT2], f32, tag="psA")
                        for kt in range(6):
                            nc.tensor.matmul(
                                ps[:tsz, :384],
                                o_sb[kt][:, c0 + t0:c0 + t0 + tsz],
                                wp_sb[kt][:, nt * 384:(nt + 1) * 384],
                                start=(kt == 0), stop=(kt == 5),
                            )
                        y_sb = cp_p.tile([128, 384], f32, tag="y")
                        nc.vector.tensor_add(
                            y_sb[:tsz], ps[:tsz, :384],
                            bias_sb[:tsz, nt * 384:(nt + 1) * 384])
                        nc.sync.dma_start(
                            out=out_d[b, t0:t0 + tsz, nt * 384:(nt + 1) * 384],
                            in_=y_sb[:tsz])

    return nc


_CACHED_NC = None


def kernel(x, w_qkv, w_proj, b_proj, factors):
    global LAST_EXEC_NS, LAST_TRACE, _CACHED_NC
    from concourse.bass_utils import run_bass_kernel_spmd

    factors = np.asarray(factors, dtype=np.float32)
    scale = HEAD_DIM ** -0.5
    G_s = _grid_g(factors) * scale
    ig = np.concatenate([np.eye(N_TOK, dtype=np.float32), G_s.T], axis=1)

    w_qkv = np.asarray(w_qkv, dtype=np.float32)
    in_common = {
        "ig": ig.astype(BF16),
        "wqT": np.ascontiguousarray(w_qkv[0:DIM, :].T).astype(BF16),
        "wkT": np.ascontiguousarray(w_qkv[DIM:2 * DIM, :].T).astype(BF16),
        "wvT": np.ascontiguousarray(w_qkv[2 * DIM:3 * DIM, :].T).astype(BF16),
        "wpT": np.ascontiguousarray(np.asarray(w_proj, dtype=np.float32).T).astype(BF16),
        "bias": np.asarray(b_proj, dtype=np.float32),
    }
    x = np.asarray(x, dtype=np.float32).astype(BF16)
    in_maps = [
        {"x": x[c * B_PER_CORE:(c + 1) * B_PER_CORE], **in_common}
        for c in range(N_CORES)
    ]

    if _CACHED_NC is None:
        _CACHED_NC = _build_bass()
    nc = _CACHED_NC

    trace = bool(int(os.environ.get("KERNEL_TRACE", "0")))
    res = run_bass_kernel_spmd(nc, in_maps, core_ids=list(range(N_CORES)),
                               trace=trace)
    LAST_EXEC_NS = res.exec_time_ns
    if res.instructions_and_trace is not None:
        LAST_TRACE = res.instructions_and_trace[1]
    out = np.concatenate([res.results[c]["out"] for c in range(N_CORES)], axis=0)
    return out.astype(np.float32)
